# revision 10
# baseline (speedup 1.0000x reference)
"""Trainium2 Bass kernel for nn_Block_56650618634972.

Math: reference = relu(AFFINE(relu(BN1(dwconv3x3(x)))))  where AFFINE is the
composition of 8 butterfly stages + per-stage BNs — all linear over the
256-channel axis — folded on host into a single 256x256 matrix M + bias.

Device work per core (batch-sharded, 4 images each):
  1. depthwise 3x3 conv: 9 diagonal-matrix matmuls (f32r) accumulating into
     PSUM over a zero-padded (58x58) SBUF copy of each 128-channel tile.
  2. conv epilogue on ScalarE: relu(psum + beta1) -> y (SBUF, f32)
  3. butterfly: dense 256x256 matmul (f32r, 2x2 blocks of 128)
  4. epilogue on VectorE: relu(psum + bias) -> out (SBUF, f32) -> DMA out
"""

import numpy as np

import concourse.bass as bass
import concourse.mybir as mybir
import concourse.tile as tile
from concourse import bacc
from concourse.bass_utils import run_bass_kernel_spmd

N_CORES = 8
IMGS = 4            # images per core (32 / 8)
C = 256
H = W = 56
L = H * W           # 3136
EPS = 1e-5
CONV_CHUNK_ROWS = 8                     # 8 rows x 56 = 448 cols <= 512 (1 PSUM bank)
CONV_CHUNK = CONV_CHUNK_ROWS * W        # 448
N_CONV_CHUNKS = H // CONV_CHUNK_ROWS    # 7
BTF_CHUNKS = [(i * 512, min(512, L - i * 512)) for i in range((L + 511) // 512)]

F32 = mybir.dt.float32
F32R = mybir.dt.float32r
RELU = mybir.ActivationFunctionType.Relu
ADD = mybir.AluOpType.add
MAX = mybir.AluOpType.max

_compiled = {}


def _build(imgs=IMGS, do_butterfly=True, do_conv=True):
    nc = bacc.Bacc("TRN2", target_bir_lowering=False, debug=False,
                   num_devices=N_CORES)
    x_d = nc.dram_tensor("x", (IMGS, C, H, W), F32, kind="ExternalInput")
    diag_d = nc.dram_tensor("diag", (2, 9, 128, 128), F32R, kind="ExternalInput")
    mt_d = nc.dram_tensor("mt", (2, 2, 128, 128), F32R, kind="ExternalInput")
    cbias_d = nc.dram_tensor("cbias", (2, 128), F32, kind="ExternalInput")
    obias_d = nc.dram_tensor("obias", (2, 128), F32, kind="ExternalInput")
    zguard_d = nc.dram_tensor("zguard", ((H + 2) * (W + 2),), F32R,
                              kind="ExternalInput")
    out_d = nc.dram_tensor("out", (IMGS, C, H, W), F32, kind="ExternalOutput")

    # per-image DRAM views: partitions = low channel, free = (ctile, spatial)
    x_v = x_d.ap().rearrange("n (t c) h w -> n c t h w", t=2)
    out_v = out_d.ap().rearrange("n (t c) h w -> n c t (h w)", t=2)

    from contextlib import ExitStack
    with tile.TileContext(nc) as tc, ExitStack() as es:
        consts = es.enter_context(tc.tile_pool(name="consts", bufs=1))
        xin_pool = es.enter_context(tc.tile_pool(name="xin", bufs=2))
        y_pool = es.enter_context(tc.tile_pool(name="ypool", bufs=2))
        o_pool = es.enter_context(tc.tile_pool(name="opool", bufs=2))
        cps_pool = es.enter_context(tc.tile_pool(name="cps", bufs=2, space="PSUM"))
        bps_pool = es.enter_context(tc.tile_pool(name="bps", bufs=2, space="PSUM"))

        # ---- constants ----
        diag_sb = consts.tile([128, 18, 128], F32R, name="diag_sb", tag="diag_sb")
        nc.sync.dma_start(out=diag_sb, in_=diag_d.ap().rearrange("a t k p -> k (a t) p"))
        mt_sb = consts.tile([128, 4, 128], F32R, name="mt_sb", tag="mt_sb")
        nc.sync.dma_start(out=mt_sb, in_=mt_d.ap().rearrange("i o k p -> k (i o) p"))
        cbias_sb = consts.tile([128, 2], F32, name="cbias_sb", tag="cbias_sb")
        nc.sync.dma_start(out=cbias_sb, in_=cbias_d.ap().rearrange("t c -> c t"))
        obias_sb = consts.tile([128, 2], F32, name="obias_sb", tag="obias_sb")
        nc.sync.dma_start(out=obias_sb, in_=obias_d.ap().rearrange("t c -> c t"))

        # persistent zero-padded x buffers (58x58 with guard ring); guards are
        # zeroed once here and never written again (pad-copy touches interior only)
        xp0 = consts.tile([128, H + 2, W + 2], F32R, name="xp0", tag="xp0")
        xp1 = consts.tile([128, H + 2, W + 2], F32R, name="xp1", tag="xp1")
        zsrc = bass.AP(tensor=zguard_d, offset=0,
                       ap=[[0, 128], [W + 2, H + 2], [1, W + 2]])
        nc.gpsimd.dma_start(out=xp0, in_=zsrc)
        nc.gpsimd.dma_start(out=xp1, in_=zsrc)
        xps = [xp0, xp1]

        for n in range(imgs):
            x_in = xin_pool.tile([128, 2, H, W], F32, tag="xin", name=f"x_in{n}")
            nc.sync.dma_start(out=x_in, in_=x_v[n])
            y_sb = y_pool.tile([128, 2, L], F32R, tag="y", name=f"y{n}")

            for ct in range(2):
                xp = xps[ct]
                # pad-copy (ScalarE): interior of the 58x58 padded layout
                nc.scalar.copy(out=xp[:, 1:57, 1:57], in_=x_in[:, ct])
                if not do_conv:
                    continue
                for cchunk in range(N_CONV_CHUNKS):
                    ps = cps_pool.tile([128, CONV_CHUNK], F32, tag="cps",
                                       name=f"cps{n}_{ct}_{cchunk}")
                    r0 = CONV_CHUNK_ROWS * cchunk
                    for t in range(9):
                        dh, dw = divmod(t, 3)
                        nc.tensor.matmul(
                            ps,
                            lhsT=diag_sb[:, ct * 9 + t, :],
                            rhs=xp[:, r0 + dh:r0 + dh + CONV_CHUNK_ROWS,
                                   dw:dw + W],
                            start=(t == 0), stop=(t == 8),
                        )
                    # epilogue: y = relu(ps + cbias[ct])
                    nc.scalar.activation(
                        out=y_sb[:, ct, CONV_CHUNK * cchunk:CONV_CHUNK * (cchunk + 1)],
                        in_=ps, func=RELU, bias=cbias_sb[:, ct:ct + 1], scale=1.0,
                    )

            o_sb = o_pool.tile([128, 2, L], F32, tag="o", name=f"o{n}")
            if not do_butterfly:
                nc.vector.tensor_copy(o_sb, y_sb.bitcast(F32))
                nc.sync.dma_start(out=out_v[n], in_=o_sb)
                continue
            for co in range(2):
                for s0, sl in BTF_CHUNKS:
                    bps = bps_pool.tile([128, 512], F32, tag="bps",
                                        name=f"bps{n}_{co}_{s0}")
                    for ci in range(2):
                        nc.tensor.matmul(
                            bps[:, :sl],
                            lhsT=mt_sb[:, ci * 2 + co, :],
                            rhs=y_sb[:, ci, s0:s0 + sl],
                            start=(ci == 0), stop=(ci == 1),
                        )
                    nc.vector.tensor_scalar(
                        o_sb[:, co, s0:s0 + sl], bps[:, :sl],
                        obias_sb[:, co:co + 1], 0.0, ADD, MAX,
                    )
            nc.sync.dma_start(out=out_v[n], in_=o_sb)

    nc.compile()
    return nc


def _fold_params(dw_w, g1, b1, m1, v1, bw, bg, bb, bm, bv):
    """Fold BN1 into conv taps; fold butterfly+BN chain into (M, bias)."""
    f8 = np.float64
    dw_w, g1, b1, m1, v1 = (np.asarray(a, f8) for a in (dw_w, g1, b1, m1, v1))
    inv1 = g1 / np.sqrt(v1 + EPS)
    cbias = b1 - m1 * inv1                       # (256,)
    w9 = dw_w[:, 0] * inv1[:, None, None]        # (256, 3, 3)

    def chain(v):
        out = np.asarray(v, f8)[None, None]      # (1, 1, 256, cols)
        for wi, gi, bi_, mi, vi in zip(bw, bg, bb, bm, bv):
            wi, gi, bi_, mi, vi = (np.asarray(a, f8) for a in (wi, gi, bi_, mi, vi))
            g = out.shape[1]
            P = out.shape[2] // 2
            Lc = out.shape[3]
            x5 = out.reshape(1, g, P, 2, Lc)
            o = np.einsum("gkq,ngpql->ngkpl", wi, x5).reshape(1, 2 * g, P, Lc)
            inv = gi / np.sqrt(vi + EPS)
            out = o * inv[None, :, None, None] + (bi_ - mi * inv)[None, :, None, None]
        return out[0].reshape(256, -1)

    obias = chain(np.zeros((256, 1)))[:, 0]      # (256,)
    M = chain(np.eye(256)) - obias[:, None]      # (256, 256)

    diag = np.zeros((2, 9, 128, 128), np.float32)
    k = np.arange(128)
    for ct in range(2):
        for t in range(9):
            dh, dw_ = divmod(t, 3)
            diag[ct, t, k, k] = w9[ct * 128 + k, dh, dw_].astype(np.float32)
    mt = M.reshape(2, 128, 2, 128).transpose(2, 0, 3, 1).astype(np.float32)
    mt = np.ascontiguousarray(mt)                # mt[ci, co, k, p] = M[co*128+p, ci*128+k]
    return (diag, mt,
            np.ascontiguousarray(cbias.reshape(2, 128).astype(np.float32)),
            np.ascontiguousarray(obias.reshape(2, 128).astype(np.float32)))


def kernel(x, dw_w, g1, b1, m1, v1, bw, bg, bb, bm, bv):
    x = np.ascontiguousarray(np.asarray(x, np.float32))
    diag, mt, cbias, obias = _fold_params(dw_w, g1, b1, m1, v1, bw, bg, bb, bm, bv)

    if "nc" not in _compiled:
        _compiled["nc"] = _build()
    nc = _compiled["nc"]

    shards = x.reshape(N_CORES, IMGS, C, H, W)
    zguard = np.zeros(((H + 2) * (W + 2),), np.float32)
    in_maps = [
        {"x": np.ascontiguousarray(shards[i]), "diag": diag, "mt": mt,
         "cbias": cbias, "obias": obias, "zguard": zguard}
        for i in range(N_CORES)
    ]
    res = run_bass_kernel_spmd(nc, in_maps, core_ids=list(range(N_CORES)))
    out = np.concatenate([res.results[i]["out"] for i in range(N_CORES)], axis=0)
    return out.reshape(32, C, H, W)
